# revision 67
# baseline (speedup 1.0000x reference)
"""Gaussian 2x2 splat (DifferentiableSquareSensor) on 8 Trainium2 NeuronCores.

Full inputs in, full 1024x1024 image out.

Math: x,y are uniform in [0,1), so pixel coords land in [512,1024) and with
sigma=0.1 every Gaussian tap except the nearest 2x2 neighborhood is <= e^-50
(~2e-22 relative) -- invisible in fp32.  The splat reduces to a separable
2x2 deposit; with only two taps per axis the normalized weights are
sigmoids: gx0' = sigmoid(50 - 100 tx), gx1' = 1 - gx0'.

Distribution: the active 512x512 region is cut into 2048 buckets (32
16-col strips x 64 8-row bands, bands offset by -1 so a 9-row tile holds
both y-taps of every owned point -- no y duplication).  Buckets are
bin-packed onto 8 cores x 256 slots (sorted deal) to equalize padding;
each core's bucket geometry arrives as per-column cb/rb tables.  On
device:
  phase A: bulk fp32 coordinate/weight math (ACT + DVE)
  phase B: per-128-point-block one-hot placement tiles built with
           broadcast-AP fp16 tensor ops (cross-bucket batched), then two
           PE matmuls per block accumulate 2x2 outer products into a
           [9, 16] PSUM tile per slot.
The kernel returns the raw per-slot tiles; the host places them into the
image (adding the one-row overlaps between vertically adjacent bands).
"""

import json
import os
import sys

import numpy as np

for _p in ("/opt/trn_rl_repo", "/root/.axon_site/_ro/trn_rl_repo"):
    if os.path.isdir(_p) and _p not in sys.path:
        sys.path.append(_p)

import concourse.bass as bass
import concourse.mybir as mybir
from concourse.bass_utils import run_bass_kernel_spmd
from concourse.tile import TileContext

P = 128
NCORES = 8
SW = 16               # strip width (cols per bucket)
BH = 8                # band height (rows per bucket)
NSX = 512 // SW       # 32 x-strips
NBAND = 512 // BH     # 64 y-bands
NGB = NSX * NBAND     # 2048 global buckets
NSLOT = NGB // NCORES  # 256 slots per core
XWIN = SW + 1         # 17: cxp1 in [0, SW], both shifted views stay inside
YWIN = BH + 2         # 10
LW = BH + 1           # 9-row tiles: bands overlap one row, folded on host
NBATCH = 192          # blocks per batched phase-B build group
CA = 512              # phase-A chunk columns (blocks)
F32 = mybir.dt.float32
F16 = mybir.dt.float16


def _split_multiwait(nc):
    """This walrus build rejects >1 sync-wait per instruction; split extras
    into single-wait NoOps placed immediately before on the same engine."""
    orig = nc.to_json_bytes

    def patched():
        js = json.loads(orig().decode())
        for fn in js["functions"]:
            for blk in fn["blocks"]:
                newlist = []
                for inst in blk["instructions"]:
                    si = inst.get("sync_info")
                    ow = (si or {}).get("on_wait") or []
                    if len(ow) > 1:
                        for k, w in enumerate(ow[:-1]):
                            newlist.append({
                                "name": f"{inst['name']}-w{k}",
                                "opcode": "NoOp",
                                "engine": inst["engine"],
                                "ins": [], "outs": [],
                                "sync_info": {"on_wait": [w], "on_update": []},
                                "bass_nofuse": True,
                            })
                        si["on_wait"] = [ow[-1]]
                    newlist.append(inst)
                blk["instructions"] = newlist
        return json.dumps(js).encode()

    nc.to_json_bytes = patched


def _slot_pos(k):
    """PSUM tile position for slot k: partitions [32(k%2), +9), columns
    [SW*((k%64)//2) + 512*(k//64), +SW)."""
    return 32 * (k % 2), SW * ((k % 64) // 2) + 512 * (k // 64)


def _build_module(nbbs):
    """SPMD bass module for per-slot block counts nbbs (tuple of NSLOT
    ints, sum even).  Blocks are slot-major; each block = 128 points on
    partitions.  Bucket geometry comes from the cb/rb input tables."""
    nbbs = np.asarray(nbbs, dtype=np.int64)
    starts = np.zeros(NSLOT + 1, dtype=np.int64)
    np.cumsum(nbbs, out=starts[1:])
    NB = int(starts[-1])
    blk_slot = np.repeat(np.arange(NSLOT), nbbs)
    blk_first = np.zeros(NB, dtype=bool)
    blk_first[starts[:-1][nbbs > 0]] = True
    blk_last = np.zeros(NB, dtype=bool)
    blk_last[starts[1:][nbbs > 0] - 1] = True

    nc = bass.Bass("TRN2", target_bir_lowering=False, debug=False,
                   num_devices=NCORES)
    xs_d = nc.dram_tensor("xs", [P, NB], F32, kind="ExternalInput")
    ys_d = nc.dram_tensor("ys", [P, NB], F32, kind="ExternalInput")
    vs_d = nc.dram_tensor("vs", [P, NB], F16, kind="ExternalInput")
    cb_d = nc.dram_tensor("cb", [1, NB], F16, kind="ExternalInput")
    rb_d = nc.dram_tensor("rb", [1, NB], F16, kind="ExternalInput")
    raw_d = nc.dram_tensor("raw", [48, 2048], F32, kind="ExternalOutput")

    # chunk boundaries: small first chunk so DVE starts quickly
    bounds = [0, min(128, NB)]
    while bounds[-1] < NB:
        bounds.append(min(bounds[-1] + CA, NB))
    chunks = list(zip(bounds[:-1], bounds[1:]))
    AF = mybir.ActivationFunctionType
    TT = mybir.AluOpType

    with TileContext(nc) as tc:
        with (
            tc.tile_pool(name="persist", bufs=1) as pers,
            tc.tile_pool(name="chunk", bufs=3) as chk,
            tc.tile_pool(name="ftmp", bufs=1) as ftmp,
            tc.tile_pool(name="batch", bufs=3) as bat,
            tc.tile_pool(name="psum", bufs=1, space="PSUM") as psp,
        ):
            # ---- one-time constants ----
            B50 = pers.tile([P, 1], F32)
            nc.vector.memset(B50[:], 50.0)
            # per-column bucket bases: cxp1 = xb - CB, ryc = yb - RB
            CB = pers.tile([P, NB], F16)
            nc.sync.dma_start(CB[:], cb_d[0:1, :].to_broadcast([P, NB]))
            RB = pers.tile([P, NB], F16)
            nc.sync.dma_start(RB[:], rb_d[0:1, :].to_broadcast([P, NB]))
            # pair-duplicated iotas: values 0,0,1,1,... so two blocks'
            # one-hots interleave in adjacent fp16 lanes (DVE 2x mode)
            XIOTA = pers.tile([P, 2 * XWIN], F16)
            nc.gpsimd.iota(XIOTA[:], pattern=[[1, XWIN], [0, 2]], base=0,
                           channel_multiplier=0,
                           allow_small_or_imprecise_dtypes=True)
            YIOTA = pers.tile([P, 2 * YWIN], F16)
            nc.gpsimd.iota(YIOTA[:], pattern=[[1, YWIN], [0, 2]], base=0,
                           channel_multiplier=0,
                           allow_small_or_imprecise_dtypes=True)

            # ---- per-point arrays, one tile per phase-A chunk so that
            # phase-B batches only depend on their own chunk (overlap) ----
            def chunk_tiles(nm):
                return [pers.tile([P, j1 - j0], F16, name=f"{nm}{i}")
                        for i, (j0, j1) in enumerate(chunks)]
            CXP1s = chunk_tiles("CXP1")
            RYCs = chunk_tiles("RYC")
            GY0s = chunk_tiles("GY0")
            A0s = chunk_tiles("A0")
            A1s = chunk_tiles("A1")

            # ---- phase A ----
            for ci, (j0, j1) in enumerate(chunks):
                C = j1 - j0
                sl = slice(j0, j1)
                X = chk.tile([P, CA], F32, name="X")
                Y = chk.tile([P, CA], F32, name="Y")
                V = chk.tile([P, CA], F16, name="V")
                nc.sync.dma_start(X[:, :C], xs_d[:, sl])
                nc.sync.dma_start(Y[:, :C], ys_d[:, sl])
                nc.sync.dma_start(V[:, :C], vs_d[:, sl])

                XP = ftmp.tile([P, CA], F32, name="XP")
                nc.scalar.activation(XP[:, :C], X[:, :C], AF.Copy,
                                     bias=512.0, scale=512.0)
                YP = ftmp.tile([P, CA], F32, name="YP")
                nc.scalar.activation(YP[:, :C], Y[:, :C], AF.Copy,
                                     bias=512.0, scale=512.0)

                # exact floor/frac: xp in [512,1024) has fp32 exponent 9, so
                # masking the low 14 mantissa bits IS floor(xp); frac exact.
                XB = ftmp.tile([P, CA], F32, name="XB")
                nc.vector.tensor_scalar(out=XB[:, :C].bitcast(mybir.dt.int32),
                                        in0=XP[:, :C].bitcast(mybir.dt.int32),
                                        scalar1=-16384, scalar2=None,
                                        op0=TT.bitwise_and)
                YB = ftmp.tile([P, CA], F32, name="YB")
                nc.vector.tensor_scalar(out=YB[:, :C].bitcast(mybir.dt.int32),
                                        in0=YP[:, :C].bitcast(mybir.dt.int32),
                                        scalar1=-16384, scalar2=None,
                                        op0=TT.bitwise_and)
                TX = ftmp.tile([P, CA], F32, name="TX")
                nc.vector.tensor_tensor(out=TX[:, :C], in0=XP[:, :C],
                                        in1=XB[:, :C], op=TT.subtract)
                TY = ftmp.tile([P, CA], F32, name="TY")
                nc.vector.tensor_tensor(out=TY[:, :C], in0=YP[:, :C],
                                        in1=YB[:, :C], op=TT.subtract)
                # only two taps matter per axis, so the normalized weights
                # are sigmoids:  gx0/(gx0+gx1) = sigmoid(50 - 100 tx), and
                # gx1' = 1 - gx0', gy1' = 1 - gy0'.  Fold v into the x pair:
                # a0 = v sigmoid(50-100 tx), a1 = v - a0; y pair stays as
                # gy0' (fp16) with T1 = YC - T0 in phase B.
                GXS = ftmp.tile([P, CA], F16, name="GXS")
                nc.scalar.activation(GXS[:, :C], TX[:, :C], AF.Sigmoid,
                                     bias=B50[:, 0:1], scale=-100.0)
                nc.scalar.activation(GY0s[ci][:, :C], TY[:, :C], AF.Sigmoid,
                                     bias=B50[:, 0:1], scale=-100.0)
                eng = nc.vector if ci == 0 else nc.gpsimd
                eng.tensor_tensor(out=A0s[ci][:, :C], in0=V[:, :C],
                                  in1=GXS[:, :C], op=TT.mult)
                eng.tensor_tensor(out=A1s[ci][:, :C], in0=V[:, :C],
                                  in1=A0s[ci][:, :C], op=TT.subtract)
                eng.tensor_tensor(out=CXP1s[ci][:, :C], in0=XB[:, :C],
                                  in1=CB[:, sl], op=TT.subtract)
                eng.tensor_tensor(out=RYCs[ci][:, :C], in0=YB[:, :C],
                                  in1=RB[:, sl], op=TT.subtract)

            # ---- phase B ----
            PS = psp.tile([P, 2048], F32)
            batches = []
            for ci, (lo, hi) in enumerate(chunks):
                j = lo
                while j < hi:
                    n = min(NBATCH, hi - j)
                    batches.append((ci, j, n))
                    j += n

            def pap(tile_ap, off, dims):
                return bass.AP(tile_ap.tensor, tile_ap.offset + off, dims)

            def stage1(ci, j0, nbt):
                """Build one-hots: XC on DVE, the y side (YC, T0) on Pool."""
                jl = j0 - chunks[ci][0]
                npair = (nbt + 1) // 2
                # paired views: element (q, f, i) = block 2q+i, window pos f
                XC = bat.tile([P, NBATCH * XWIN], F16, name="XC")
                nc.vector.tensor_tensor(
                    out=pap(XC[:], 0, [XC[:].ap[0], [2 * XWIN, npair], [2, XWIN], [1, 2]]),
                    in0=pap(XIOTA[:], 0, [XIOTA[:].ap[0], [0, npair], [2, XWIN], [1, 2]]),
                    in1=pap(CXP1s[ci][:], jl, [CXP1s[ci][:].ap[0], [2, npair], [0, XWIN], [1, 2]]),
                    op=TT.is_equal)
                YC = bat.tile([P, NBATCH * YWIN], F16, name="YC")
                nc.vector.tensor_tensor(
                    out=pap(YC[:], 0, [YC[:].ap[0], [2 * YWIN, npair], [2, YWIN], [1, 2]]),
                    in0=pap(YIOTA[:], 0, [YIOTA[:].ap[0], [0, npair], [2, YWIN], [1, 2]]),
                    in1=pap(RYCs[ci][:], jl, [RYCs[ci][:].ap[0], [2, npair], [0, YWIN], [1, 2]]),
                    op=TT.is_equal)
                T0 = bat.tile([P, NBATCH * YWIN], F16, name="T0")
                nc.gpsimd.tensor_tensor(
                    out=pap(T0[:], 0, [T0[:].ap[0], [2 * YWIN, npair], [2, YWIN], [1, 2]]),
                    in0=pap(YC[:], 0, [YC[:].ap[0], [2 * YWIN, npair], [2, YWIN], [1, 2]]),
                    in1=pap(GY0s[ci][:], jl, [GY0s[ci][:].ap[0], [2, npair], [0, YWIN], [1, 2]]),
                    op=TT.mult)
                return XC, YC, T0

            def stage2(ci, j0, nbt, XC, YC, T0):
                """Combine row weights and run the per-block matmuls."""
                jl = j0 - chunks[ci][0]
                npair = (nbt + 1) // 2
                # L[k, r] = gy0*(r+1==ryc) + gy1*(r==ryc)
                #         = (T0[r+1] - T0[r]) + YC[r]   since gy1 = 1 - gy0
                D = bat.tile([P, NBATCH * LW], F16, name="D")
                nc.vector.tensor_tensor(
                    out=pap(D[:], 0, [D[:].ap[0], [2 * LW, npair], [2, LW], [1, 2]]),
                    in0=pap(T0[:], 2, [T0[:].ap[0], [2 * YWIN, npair], [2, LW], [1, 2]]),
                    in1=pap(T0[:], 0, [T0[:].ap[0], [2 * YWIN, npair], [2, LW], [1, 2]]),
                    op=TT.subtract)
                L = bat.tile([P, NBATCH * LW], F16, name="L")
                nc.vector.tensor_tensor(
                    out=pap(L[:], 0, [L[:].ap[0], [2 * LW, npair], [2, LW], [1, 2]]),
                    in0=pap(D[:], 0, [D[:].ap[0], [2 * LW, npair], [2, LW], [1, 2]]),
                    in1=pap(YC[:], 0, [YC[:].ap[0], [2 * YWIN, npair], [2, LW], [1, 2]]),
                    op=TT.add)
                LA0 = bat.tile([P, NBATCH * LW], F16, name="LA0")
                nc.vector.tensor_tensor(
                    out=pap(LA0[:], 0, [LA0[:].ap[0], [2 * LW, npair], [2, LW], [1, 2]]),
                    in0=pap(L[:], 0, [L[:].ap[0], [2 * LW, npair], [2, LW], [1, 2]]),
                    in1=pap(A0s[ci][:], jl, [A0s[ci][:].ap[0], [2, npair], [0, LW], [1, 2]]),
                    op=TT.mult)
                LA1 = bat.tile([P, NBATCH * LW], F16, name="LA1")
                nc.vector.tensor_tensor(
                    out=pap(LA1[:], 0, [LA1[:].ap[0], [2 * LW, npair], [2, LW], [1, 2]]),
                    in0=pap(L[:], 0, [L[:].ap[0], [2 * LW, npair], [2, LW], [1, 2]]),
                    in1=pap(A1s[ci][:], jl, [A1s[ci][:].ap[0], [2, npair], [0, LW], [1, 2]]),
                    op=TT.mult)

                for b in range(nbt):
                    q, i = b // 2, b % 2
                    g = j0 + b          # global block
                    prow, pcol = _slot_pos(int(blk_slot[g]))
                    out_ap = PS[prow:prow + LW, pcol:pcol + SW]
                    lhsT0 = pap(LA0[:], q * 2 * LW + i, [LA0[:].ap[0], [2, LW]])
                    lhsT1 = pap(LA1[:], q * 2 * LW + i, [LA1[:].ap[0], [2, LW]])
                    rhs0 = pap(XC[:], q * 2 * XWIN + i + 2, [XC[:].ap[0], [2, SW]])
                    rhs1 = pap(XC[:], q * 2 * XWIN + i, [XC[:].ap[0], [2, SW]])
                    nc.tensor.matmul(out=out_ap, lhsT=lhsT0, rhs=rhs0,
                                     start=bool(blk_first[g]), stop=False)
                    nc.tensor.matmul(out=out_ap, lhsT=lhsT1, rhs=rhs1,
                                     start=False, stop=bool(blk_last[g]))

            # ---- writeback: raw per-slot tiles, host does placement ----
            OUT = pers.tile([P, 2048], F32)

            def writeback(q):
                csl = slice(512 * q, 512 * q + 512)
                nc.scalar.activation(OUT[0:48, csl], PS[0:48, csl], AF.Copy)
                nc.sync.dma_start(raw_d[0:48, csl], OUT[0:48, csl])

            # software-pipeline: batch k's stage1 runs (DVE) while batch
            # k-1's Pool T1 completes, so L never stalls the in-order DVE
            # queue behind the Pool op.  Each PSUM quarter's writeback is
            # emitted one batch after its last matmul to overlap phase B.
            trig = {}
            for q in range(3):
                qend = int(starts[64 * (q + 1)])
                t = next(i for i, (_, j0, nbt) in enumerate(batches)
                         if j0 + nbt >= qend)
                trig.setdefault(t + 1, []).append(q)
            written = set()
            prev = None
            for k, (ci, j0, nbt) in enumerate(batches):
                tiles = stage1(ci, j0, nbt)
                if prev is not None:
                    pk, (pci, pj0, pnbt), (pXC, pYC, pT0) = prev
                    stage2(pci, pj0, pnbt, pXC, pYC, pT0)
                    for q in trig.get(pk, []):
                        writeback(q)
                        written.add(q)
                prev = (k, (ci, j0, nbt), tiles)
            pk, (pci, pj0, pnbt), (pXC, pYC, pT0) = prev
            stage2(pci, pj0, pnbt, pXC, pYC, pT0)
            for q in range(4):
                if q not in written:
                    writeback(q)

    _split_multiwait(nc)
    return nc


def _shard(x, y, v):
    """Host sharding: assign each point (+x-boundary duplicates) to a
    global (strip, band) bucket, bin-pack buckets onto cores/slots, and
    build the padded per-core input arrays + geometry tables."""
    xp = (x + np.float32(1.0)) * np.float32(512.0)
    yp = (y + np.float32(1.0)) * np.float32(512.0)
    cx = np.floor(xp).astype(np.int32) - 512          # 0..511
    cy = np.floor(yp).astype(np.int32) - 512

    def assign(cx, cy):
        strip = np.clip(cx, 0, 511) // SW
        # band w owns cy in [BH w - 1, BH w + BH - 1); its 9-row tile
        # holds both y-taps, so no y-duplication is needed.
        band = np.clip((cy + 1) // BH, 0, NBAND - 1)
        return strip * NBAND + band

    xdup = ((cx & (SW - 1)) == SW - 1) & (cx != 511)

    idx = np.arange(x.shape[0], dtype=np.int64)
    all_idx = np.concatenate([idx, idx[xdup]])
    key = np.concatenate([assign(cx, cy), assign(cx[xdup] + 1, cy[xdup])])

    order = np.argsort(key, kind="stable")
    all_idx = all_idx[order]
    counts = np.bincount(key, minlength=NGB)
    gstarts = np.zeros(NGB + 1, dtype=np.int64)
    np.cumsum(counts, out=gstarts[1:])

    # sorted deal: slot k of every core gets similarly sized buckets
    perm = np.argsort(-counts, kind="stable")       # bucket ids, big first
    slot_bkt = perm.reshape(NSLOT, NCORES)          # [slot, core] -> bucket
    slot_cnt = counts[slot_bkt]                     # [slot, core]
    nbbs = np.maximum(-(-slot_cnt.max(axis=1) // P), 1)
    if nbbs.sum() % 2:
        nbbs[-1] += 1
    sstarts = np.zeros(NSLOT + 1, dtype=np.int64)
    np.cumsum(nbbs, out=sstarts[1:])
    NB = int(sstarts[-1])
    slot = NB * P

    per_core = []
    slotmap = np.zeros((NCORES, NSLOT, 2), dtype=np.int32)  # (strip, band)
    for c in range(NCORES):
        xs = np.full(slot, 0.25, dtype=np.float32)
        ys = np.full(slot, 0.25, dtype=np.float32)
        vs = np.zeros(slot, dtype=np.float16)
        cb = np.zeros(NB, dtype=np.float16)
        rb = np.zeros(NB, dtype=np.float16)
        for k in range(NSLOT):
            bkt = int(slot_bkt[k, c])
            strip, band = bkt // NBAND, bkt % NBAND
            slotmap[c, k] = (strip, band)
            seg = all_idx[gstarts[bkt]:gstarts[bkt + 1]]
            off = int(sstarts[k]) * P
            xs[off:off + seg.size] = x[seg]
            ys[off:off + seg.size] = y[seg]
            vs[off:off + seg.size] = v[seg]
            cols = slice(int(sstarts[k]), int(sstarts[k + 1]))
            cb[cols] = 511.0 + SW * strip
            rb[cols] = 510.0 + BH * band
        per_core.append({
            "xs": np.ascontiguousarray(xs.reshape(NB, P).T),
            "ys": np.ascontiguousarray(ys.reshape(NB, P).T),
            "vs": np.ascontiguousarray(vs.reshape(NB, P).T),
            "cb": cb[None, :],
            "rb": rb[None, :],
        })
    return per_core, tuple(int(n) for n in nbbs), slotmap


_CACHE = {}


def kernel(x, y, values):
    x = np.asarray(x, dtype=np.float32)
    y = np.asarray(y, dtype=np.float32)
    v = np.asarray(values, dtype=np.float32)

    per_core, nbbs, slotmap = _shard(x, y, v)
    if nbbs not in _CACHE:
        _CACHE[nbbs] = _build_module(nbbs)
    nc = _CACHE[nbbs]

    res = run_bass_kernel_spmd(nc, per_core, core_ids=list(range(NCORES)))

    img = np.zeros((1024, 1024), dtype=np.float32)
    act = img[512:, 512:]
    for c in range(NCORES):
        raw = res.results[c]["raw"]
        for k in range(NSLOT):
            strip, band = slotmap[c, k]
            prow, pcol = _slot_pos(k)
            tile = raw[prow:prow + LW, pcol:pcol + SW]
            r0 = BH * band - 1
            lo = 1 if r0 < 0 else 0
            act[r0 + lo:r0 + LW, SW * strip:SW * strip + SW] += tile[lo:]
    return img


# revision 68
# speedup vs baseline: 1.0086x; 1.0086x over previous
"""Gaussian 2x2 splat (DifferentiableSquareSensor) on 8 Trainium2 NeuronCores.

Full inputs in, full 1024x1024 image out.

Math: x,y are uniform in [0,1), so pixel coords land in [512,1024) and with
sigma=0.1 every Gaussian tap except the nearest 2x2 neighborhood is <= e^-50
(~2e-22 relative) -- invisible in fp32.  The splat reduces to a separable
2x2 deposit; with only two taps per axis the normalized weights are
sigmoids: gx0' = sigmoid(50 - 100 tx), gx1' = 1 - gx0'.

Distribution: the active 512x512 region is cut into 2048 buckets (32
16-col strips x 64 8-row bands, bands offset by -1 so a 9-row tile holds
both y-taps of every owned point -- no y duplication).  Buckets are
bin-packed onto 8 cores x 256 slots (sorted deal) to equalize padding;
each core's bucket geometry arrives as per-column cb/rb tables.  On
device:
  phase A: bulk fp32 coordinate/weight math (ACT + DVE)
  phase B: per-128-point-block one-hot placement tiles built with
           broadcast-AP fp16 tensor ops (cross-bucket batched), then two
           PE matmuls per block accumulate 2x2 outer products into a
           [9, 16] PSUM tile per slot.
The kernel returns the raw per-slot tiles; the host places them into the
image (adding the one-row overlaps between vertically adjacent bands).
"""

import json
import os
import sys

import numpy as np

for _p in ("/opt/trn_rl_repo", "/root/.axon_site/_ro/trn_rl_repo"):
    if os.path.isdir(_p) and _p not in sys.path:
        sys.path.append(_p)

import concourse.bass as bass
import concourse.mybir as mybir
from concourse.bass_utils import run_bass_kernel_spmd
from concourse.tile import TileContext

P = 128
NCORES = 8
SW = 16               # strip width (cols per bucket)
BH = 8                # band height (rows per bucket)
NSX = 512 // SW       # 32 x-strips
NBAND = 512 // BH     # 64 y-bands
NGB = NSX * NBAND     # 2048 global buckets
NSLOT = NGB // NCORES  # 256 slots per core
XWIN = SW + 1         # 17: cxp1 in [0, SW], both shifted views stay inside
YWIN = BH + 2         # 10
LW = BH + 1           # 9-row tiles: bands overlap one row, folded on host
NBATCH = 192          # blocks per batched phase-B build group
CA = 512              # phase-A chunk columns (blocks)
F32 = mybir.dt.float32
F16 = mybir.dt.float16


def _split_multiwait(nc):
    """This walrus build rejects >1 sync-wait per instruction; split extras
    into single-wait NoOps placed immediately before on the same engine."""
    orig = nc.to_json_bytes

    def patched():
        js = json.loads(orig().decode())
        for fn in js["functions"]:
            for blk in fn["blocks"]:
                newlist = []
                for inst in blk["instructions"]:
                    si = inst.get("sync_info")
                    ow = (si or {}).get("on_wait") or []
                    if len(ow) > 1:
                        for k, w in enumerate(ow[:-1]):
                            newlist.append({
                                "name": f"{inst['name']}-w{k}",
                                "opcode": "NoOp",
                                "engine": inst["engine"],
                                "ins": [], "outs": [],
                                "sync_info": {"on_wait": [w], "on_update": []},
                                "bass_nofuse": True,
                            })
                        si["on_wait"] = [ow[-1]]
                    newlist.append(inst)
                blk["instructions"] = newlist
        return json.dumps(js).encode()

    nc.to_json_bytes = patched


def _slot_pos(k):
    """PSUM tile position for slot k: partitions [32(k%2), +9), columns
    [SW*((k%64)//2) + 512*(k//64), +SW)."""
    return 32 * (k % 2), SW * ((k % 64) // 2) + 512 * (k // 64)


def _build_module(nbbs):
    """SPMD bass module for per-slot block counts nbbs (tuple of NSLOT
    ints, sum even).  Blocks are slot-major; each block = 128 points on
    partitions.  Bucket geometry comes from the cb/rb input tables."""
    nbbs = np.asarray(nbbs, dtype=np.int64)
    starts = np.zeros(NSLOT + 1, dtype=np.int64)
    np.cumsum(nbbs, out=starts[1:])
    NB = int(starts[-1])
    blk_slot = np.repeat(np.arange(NSLOT), nbbs)
    blk_first = np.zeros(NB, dtype=bool)
    blk_first[starts[:-1][nbbs > 0]] = True
    blk_last = np.zeros(NB, dtype=bool)
    blk_last[starts[1:][nbbs > 0] - 1] = True

    nc = bass.Bass("TRN2", target_bir_lowering=False, debug=False,
                   num_devices=NCORES)
    xs_d = nc.dram_tensor("xs", [P, NB], F32, kind="ExternalInput")
    ys_d = nc.dram_tensor("ys", [P, NB], F32, kind="ExternalInput")
    vs_d = nc.dram_tensor("vs", [P, NB], F16, kind="ExternalInput")
    cb_d = nc.dram_tensor("cb", [1, NB], F16, kind="ExternalInput")
    rb_d = nc.dram_tensor("rb", [1, NB], F16, kind="ExternalInput")
    raw_d = nc.dram_tensor("raw", [48, 2048], F32, kind="ExternalOutput")

    # chunk boundaries: small first chunk so DVE starts quickly
    bounds = [0, min(128, NB)]
    while bounds[-1] < NB:
        bounds.append(min(bounds[-1] + CA, NB))
    chunks = list(zip(bounds[:-1], bounds[1:]))
    AF = mybir.ActivationFunctionType
    TT = mybir.AluOpType

    with TileContext(nc) as tc:
        with (
            tc.tile_pool(name="persist", bufs=1) as pers,
            tc.tile_pool(name="chunk", bufs=3) as chk,
            tc.tile_pool(name="ftmp", bufs=1) as ftmp,
            tc.tile_pool(name="batch", bufs=3) as bat,
            tc.tile_pool(name="psum", bufs=1, space="PSUM") as psp,
        ):
            # ---- one-time constants ----
            B50 = pers.tile([P, 1], F32)
            nc.vector.memset(B50[:], 50.0)
            # per-column bucket bases: cxp1 = xb - CB, ryc = yb - RB
            CB = pers.tile([P, NB], F16)
            nc.gpsimd.dma_start(CB[:], cb_d[0:1, :].to_broadcast([P, NB]))
            RB = pers.tile([P, NB], F16)
            nc.gpsimd.dma_start(RB[:], rb_d[0:1, :].to_broadcast([P, NB]))
            # pair-duplicated iotas: values 0,0,1,1,... so two blocks'
            # one-hots interleave in adjacent fp16 lanes (DVE 2x mode)
            XIOTA = pers.tile([P, 2 * XWIN], F16)
            nc.gpsimd.iota(XIOTA[:], pattern=[[1, XWIN], [0, 2]], base=0,
                           channel_multiplier=0,
                           allow_small_or_imprecise_dtypes=True)
            YIOTA = pers.tile([P, 2 * YWIN], F16)
            nc.gpsimd.iota(YIOTA[:], pattern=[[1, YWIN], [0, 2]], base=0,
                           channel_multiplier=0,
                           allow_small_or_imprecise_dtypes=True)

            # ---- per-point arrays, one tile per phase-A chunk so that
            # phase-B batches only depend on their own chunk (overlap) ----
            def chunk_tiles(nm):
                return [pers.tile([P, j1 - j0], F16, name=f"{nm}{i}")
                        for i, (j0, j1) in enumerate(chunks)]
            CXP1s = chunk_tiles("CXP1")
            RYCs = chunk_tiles("RYC")
            GY0s = chunk_tiles("GY0")
            A0s = chunk_tiles("A0")
            A1s = chunk_tiles("A1")

            # ---- phase A ----
            for ci, (j0, j1) in enumerate(chunks):
                C = j1 - j0
                sl = slice(j0, j1)
                X = chk.tile([P, CA], F32, name="X")
                Y = chk.tile([P, CA], F32, name="Y")
                V = chk.tile([P, CA], F16, name="V")
                nc.sync.dma_start(X[:, :C], xs_d[:, sl])
                nc.sync.dma_start(Y[:, :C], ys_d[:, sl])
                nc.sync.dma_start(V[:, :C], vs_d[:, sl])

                XP = ftmp.tile([P, CA], F32, name="XP")
                nc.scalar.activation(XP[:, :C], X[:, :C], AF.Copy,
                                     bias=512.0, scale=512.0)
                YP = ftmp.tile([P, CA], F32, name="YP")
                nc.scalar.activation(YP[:, :C], Y[:, :C], AF.Copy,
                                     bias=512.0, scale=512.0)

                # exact floor/frac: xp in [512,1024) has fp32 exponent 9, so
                # masking the low 14 mantissa bits IS floor(xp); frac exact.
                XB = ftmp.tile([P, CA], F32, name="XB")
                nc.vector.tensor_scalar(out=XB[:, :C].bitcast(mybir.dt.int32),
                                        in0=XP[:, :C].bitcast(mybir.dt.int32),
                                        scalar1=-16384, scalar2=None,
                                        op0=TT.bitwise_and)
                YB = ftmp.tile([P, CA], F32, name="YB")
                nc.vector.tensor_scalar(out=YB[:, :C].bitcast(mybir.dt.int32),
                                        in0=YP[:, :C].bitcast(mybir.dt.int32),
                                        scalar1=-16384, scalar2=None,
                                        op0=TT.bitwise_and)
                TX = ftmp.tile([P, CA], F32, name="TX")
                nc.vector.tensor_tensor(out=TX[:, :C], in0=XP[:, :C],
                                        in1=XB[:, :C], op=TT.subtract)
                TY = ftmp.tile([P, CA], F32, name="TY")
                nc.vector.tensor_tensor(out=TY[:, :C], in0=YP[:, :C],
                                        in1=YB[:, :C], op=TT.subtract)
                # only two taps matter per axis, so the normalized weights
                # are sigmoids:  gx0/(gx0+gx1) = sigmoid(50 - 100 tx), and
                # gx1' = 1 - gx0', gy1' = 1 - gy0'.  Fold v into the x pair:
                # a0 = v sigmoid(50-100 tx), a1 = v - a0; y pair stays as
                # gy0' (fp16) with T1 = YC - T0 in phase B.
                GXS = ftmp.tile([P, CA], F16, name="GXS")
                nc.scalar.activation(GXS[:, :C], TX[:, :C], AF.Sigmoid,
                                     bias=B50[:, 0:1], scale=-100.0)
                nc.scalar.activation(GY0s[ci][:, :C], TY[:, :C], AF.Sigmoid,
                                     bias=B50[:, 0:1], scale=-100.0)
                eng = nc.vector if ci == 0 else nc.gpsimd
                eng.tensor_tensor(out=A0s[ci][:, :C], in0=V[:, :C],
                                  in1=GXS[:, :C], op=TT.mult)
                eng.tensor_tensor(out=A1s[ci][:, :C], in0=V[:, :C],
                                  in1=A0s[ci][:, :C], op=TT.subtract)
                eng.tensor_tensor(out=CXP1s[ci][:, :C], in0=XB[:, :C],
                                  in1=CB[:, sl], op=TT.subtract)
                eng.tensor_tensor(out=RYCs[ci][:, :C], in0=YB[:, :C],
                                  in1=RB[:, sl], op=TT.subtract)

            # ---- phase B ----
            PS = psp.tile([P, 2048], F32)
            batches = []
            for ci, (lo, hi) in enumerate(chunks):
                j = lo
                while j < hi:
                    n = min(NBATCH, hi - j)
                    batches.append((ci, j, n))
                    j += n

            def pap(tile_ap, off, dims):
                return bass.AP(tile_ap.tensor, tile_ap.offset + off, dims)

            def stage1(ci, j0, nbt):
                """Build one-hots: XC on DVE, the y side (YC, T0) on Pool."""
                jl = j0 - chunks[ci][0]
                npair = (nbt + 1) // 2
                # paired views: element (q, f, i) = block 2q+i, window pos f
                XC = bat.tile([P, NBATCH * XWIN], F16, name="XC")
                nc.vector.tensor_tensor(
                    out=pap(XC[:], 0, [XC[:].ap[0], [2 * XWIN, npair], [2, XWIN], [1, 2]]),
                    in0=pap(XIOTA[:], 0, [XIOTA[:].ap[0], [0, npair], [2, XWIN], [1, 2]]),
                    in1=pap(CXP1s[ci][:], jl, [CXP1s[ci][:].ap[0], [2, npair], [0, XWIN], [1, 2]]),
                    op=TT.is_equal)
                YC = bat.tile([P, NBATCH * YWIN], F16, name="YC")
                nc.vector.tensor_tensor(
                    out=pap(YC[:], 0, [YC[:].ap[0], [2 * YWIN, npair], [2, YWIN], [1, 2]]),
                    in0=pap(YIOTA[:], 0, [YIOTA[:].ap[0], [0, npair], [2, YWIN], [1, 2]]),
                    in1=pap(RYCs[ci][:], jl, [RYCs[ci][:].ap[0], [2, npair], [0, YWIN], [1, 2]]),
                    op=TT.is_equal)
                T0 = bat.tile([P, NBATCH * YWIN], F16, name="T0")
                nc.gpsimd.tensor_tensor(
                    out=pap(T0[:], 0, [T0[:].ap[0], [2 * YWIN, npair], [2, YWIN], [1, 2]]),
                    in0=pap(YC[:], 0, [YC[:].ap[0], [2 * YWIN, npair], [2, YWIN], [1, 2]]),
                    in1=pap(GY0s[ci][:], jl, [GY0s[ci][:].ap[0], [2, npair], [0, YWIN], [1, 2]]),
                    op=TT.mult)
                return XC, YC, T0

            def stage2(ci, j0, nbt, XC, YC, T0):
                """Combine row weights and run the per-block matmuls."""
                jl = j0 - chunks[ci][0]
                npair = (nbt + 1) // 2
                # L[k, r] = gy0*(r+1==ryc) + gy1*(r==ryc)
                #         = (T0[r+1] - T0[r]) + YC[r]   since gy1 = 1 - gy0
                D = bat.tile([P, NBATCH * LW], F16, name="D")
                nc.vector.tensor_tensor(
                    out=pap(D[:], 0, [D[:].ap[0], [2 * LW, npair], [2, LW], [1, 2]]),
                    in0=pap(T0[:], 2, [T0[:].ap[0], [2 * YWIN, npair], [2, LW], [1, 2]]),
                    in1=pap(T0[:], 0, [T0[:].ap[0], [2 * YWIN, npair], [2, LW], [1, 2]]),
                    op=TT.subtract)
                L = bat.tile([P, NBATCH * LW], F16, name="L")
                nc.vector.tensor_tensor(
                    out=pap(L[:], 0, [L[:].ap[0], [2 * LW, npair], [2, LW], [1, 2]]),
                    in0=pap(D[:], 0, [D[:].ap[0], [2 * LW, npair], [2, LW], [1, 2]]),
                    in1=pap(YC[:], 0, [YC[:].ap[0], [2 * YWIN, npair], [2, LW], [1, 2]]),
                    op=TT.add)
                LA0 = bat.tile([P, NBATCH * LW], F16, name="LA0")
                nc.vector.tensor_tensor(
                    out=pap(LA0[:], 0, [LA0[:].ap[0], [2 * LW, npair], [2, LW], [1, 2]]),
                    in0=pap(L[:], 0, [L[:].ap[0], [2 * LW, npair], [2, LW], [1, 2]]),
                    in1=pap(A0s[ci][:], jl, [A0s[ci][:].ap[0], [2, npair], [0, LW], [1, 2]]),
                    op=TT.mult)
                LA1 = bat.tile([P, NBATCH * LW], F16, name="LA1")
                nc.vector.tensor_tensor(
                    out=pap(LA1[:], 0, [LA1[:].ap[0], [2 * LW, npair], [2, LW], [1, 2]]),
                    in0=pap(L[:], 0, [L[:].ap[0], [2 * LW, npair], [2, LW], [1, 2]]),
                    in1=pap(A1s[ci][:], jl, [A1s[ci][:].ap[0], [2, npair], [0, LW], [1, 2]]),
                    op=TT.mult)

                for b in range(nbt):
                    q, i = b // 2, b % 2
                    g = j0 + b          # global block
                    prow, pcol = _slot_pos(int(blk_slot[g]))
                    out_ap = PS[prow:prow + LW, pcol:pcol + SW]
                    lhsT0 = pap(LA0[:], q * 2 * LW + i, [LA0[:].ap[0], [2, LW]])
                    lhsT1 = pap(LA1[:], q * 2 * LW + i, [LA1[:].ap[0], [2, LW]])
                    rhs0 = pap(XC[:], q * 2 * XWIN + i + 2, [XC[:].ap[0], [2, SW]])
                    rhs1 = pap(XC[:], q * 2 * XWIN + i, [XC[:].ap[0], [2, SW]])
                    nc.tensor.matmul(out=out_ap, lhsT=lhsT0, rhs=rhs0,
                                     start=bool(blk_first[g]), stop=False)
                    nc.tensor.matmul(out=out_ap, lhsT=lhsT1, rhs=rhs1,
                                     start=False, stop=bool(blk_last[g]))

            # ---- writeback: raw per-slot tiles, host does placement ----
            OUT = pers.tile([P, 2048], F32)

            def writeback(q):
                csl = slice(512 * q, 512 * q + 512)
                nc.scalar.activation(OUT[0:48, csl], PS[0:48, csl], AF.Copy)
                nc.sync.dma_start(raw_d[0:48, csl], OUT[0:48, csl])

            # software-pipeline: batch k's stage1 runs (DVE) while batch
            # k-1's Pool T1 completes, so L never stalls the in-order DVE
            # queue behind the Pool op.  Each PSUM quarter's writeback is
            # emitted one batch after its last matmul to overlap phase B.
            trig = {}
            for q in range(3):
                qend = int(starts[64 * (q + 1)])
                t = next(i for i, (_, j0, nbt) in enumerate(batches)
                         if j0 + nbt >= qend)
                trig.setdefault(t + 1, []).append(q)
            written = set()
            prev = None
            for k, (ci, j0, nbt) in enumerate(batches):
                tiles = stage1(ci, j0, nbt)
                if prev is not None:
                    pk, (pci, pj0, pnbt), (pXC, pYC, pT0) = prev
                    stage2(pci, pj0, pnbt, pXC, pYC, pT0)
                    for q in trig.get(pk, []):
                        writeback(q)
                        written.add(q)
                prev = (k, (ci, j0, nbt), tiles)
            pk, (pci, pj0, pnbt), (pXC, pYC, pT0) = prev
            stage2(pci, pj0, pnbt, pXC, pYC, pT0)
            for q in range(4):
                if q not in written:
                    writeback(q)

    _split_multiwait(nc)
    return nc


def _shard(x, y, v):
    """Host sharding: assign each point (+x-boundary duplicates) to a
    global (strip, band) bucket, bin-pack buckets onto cores/slots, and
    build the padded per-core input arrays + geometry tables."""
    xp = (x + np.float32(1.0)) * np.float32(512.0)
    yp = (y + np.float32(1.0)) * np.float32(512.0)
    cx = np.floor(xp).astype(np.int32) - 512          # 0..511
    cy = np.floor(yp).astype(np.int32) - 512

    def assign(cx, cy):
        strip = np.clip(cx, 0, 511) // SW
        # band w owns cy in [BH w - 1, BH w + BH - 1); its 9-row tile
        # holds both y-taps, so no y-duplication is needed.
        band = np.clip((cy + 1) // BH, 0, NBAND - 1)
        return strip * NBAND + band

    xdup = ((cx & (SW - 1)) == SW - 1) & (cx != 511)

    idx = np.arange(x.shape[0], dtype=np.int64)
    all_idx = np.concatenate([idx, idx[xdup]])
    key = np.concatenate([assign(cx, cy), assign(cx[xdup] + 1, cy[xdup])])

    order = np.argsort(key, kind="stable")
    all_idx = all_idx[order]
    counts = np.bincount(key, minlength=NGB)
    gstarts = np.zeros(NGB + 1, dtype=np.int64)
    np.cumsum(counts, out=gstarts[1:])

    # sorted deal: slot k of every core gets similarly sized buckets
    perm = np.argsort(-counts, kind="stable")       # bucket ids, big first
    slot_bkt = perm.reshape(NSLOT, NCORES)          # [slot, core] -> bucket
    slot_cnt = counts[slot_bkt]                     # [slot, core]
    nbbs = np.maximum(-(-slot_cnt.max(axis=1) // P), 1)
    if nbbs.sum() % 2:
        nbbs[-1] += 1
    sstarts = np.zeros(NSLOT + 1, dtype=np.int64)
    np.cumsum(nbbs, out=sstarts[1:])
    NB = int(sstarts[-1])
    slot = NB * P

    per_core = []
    slotmap = np.zeros((NCORES, NSLOT, 2), dtype=np.int32)  # (strip, band)
    for c in range(NCORES):
        xs = np.full(slot, 0.25, dtype=np.float32)
        ys = np.full(slot, 0.25, dtype=np.float32)
        vs = np.zeros(slot, dtype=np.float16)
        cb = np.zeros(NB, dtype=np.float16)
        rb = np.zeros(NB, dtype=np.float16)
        for k in range(NSLOT):
            bkt = int(slot_bkt[k, c])
            strip, band = bkt // NBAND, bkt % NBAND
            slotmap[c, k] = (strip, band)
            seg = all_idx[gstarts[bkt]:gstarts[bkt + 1]]
            off = int(sstarts[k]) * P
            xs[off:off + seg.size] = x[seg]
            ys[off:off + seg.size] = y[seg]
            vs[off:off + seg.size] = v[seg]
            cols = slice(int(sstarts[k]), int(sstarts[k + 1]))
            cb[cols] = 511.0 + SW * strip
            rb[cols] = 510.0 + BH * band
        per_core.append({
            "xs": np.ascontiguousarray(xs.reshape(NB, P).T),
            "ys": np.ascontiguousarray(ys.reshape(NB, P).T),
            "vs": np.ascontiguousarray(vs.reshape(NB, P).T),
            "cb": cb[None, :],
            "rb": rb[None, :],
        })
    return per_core, tuple(int(n) for n in nbbs), slotmap


_CACHE = {}


def kernel(x, y, values):
    x = np.asarray(x, dtype=np.float32)
    y = np.asarray(y, dtype=np.float32)
    v = np.asarray(values, dtype=np.float32)

    per_core, nbbs, slotmap = _shard(x, y, v)
    if nbbs not in _CACHE:
        _CACHE[nbbs] = _build_module(nbbs)
    nc = _CACHE[nbbs]

    res = run_bass_kernel_spmd(nc, per_core, core_ids=list(range(NCORES)))

    img = np.zeros((1024, 1024), dtype=np.float32)
    act = img[512:, 512:]
    for c in range(NCORES):
        raw = res.results[c]["raw"]
        for k in range(NSLOT):
            strip, band = slotmap[c, k]
            prow, pcol = _slot_pos(k)
            tile = raw[prow:prow + LW, pcol:pcol + SW]
            r0 = BH * band - 1
            lo = 1 if r0 < 0 else 0
            act[r0 + lo:r0 + LW, SW * strip:SW * strip + SW] += tile[lo:]
    return img
